# revision 19
# baseline (speedup 1.0000x reference)
"""Trainium2 Bass kernel for nn_Loop_Projection (batched per-prototype GEMM).

Computes out[b, e, p] = sum_d x[b, d, p] * W[p, d, e] + b[p, e] with
x: [256, 512, 128] f32, W: [128, 512, 128] f32, b: [128, 128] f32.

Sharding: prototype axis P=128 split across 8 NeuronCores (16 protos each).
Inputs are cast to fp16 on the host (well-scaled data: x ~ N(0,1),
W ~ U(+-0.107)), which halves HBM load traffic vs f32 and runs the PE at
full rate. Per proto, x and W slices are packed into ONE contiguous
[128, 1536] fp16 block, chunk-major so a DMA prefix is immediately usable:
  xw[p][k, 384c + b]       = x[b, 128c + k, p]     (b in [0,256))
  xw[p][k, 384c + 256 + e] = W[p, 128c + k, e]     (e in [0,128))

Schedule (per core, raw bacc, hand-placed semaphores):
- Loads ride the two HWDGE rings (SP=sync: odd protos, Act=scalar: even).
  DMA_DIRECT2D issue costs ~650 ns regardless of size, so mid-stream protos
  load as single 384 KiB DMAs. The first proto is split (96 KiB head on
  sync + 288 KiB on scalar) so the PE's first matmul starts ~2 us earlier,
  and the last three protos are split into halves/quarters so the PE tail
  tracks the final chunk arrivals instead of trailing whole-tile waits.
- Per proto the PE accumulates out.T = W_p.T @ x_p.T ([E, B] fp32 PSUM)
  over the 4 K-chunks; the vector engine adds bias during the PSUM->SBUF
  copy (casting to fp16) into one [E, PL*B] strip.
- Stores are DEFERRED until both load rings drain (loads and stores share
  the ~410 GB/s DMA-engine aggregate, so mid-stream stores push the
  critical last load later): protos 0-3/4-7/8-11 as three grouped DMAs on
  the gpsimd SWDGE ring, 12-13 on sync, 14 on scalar, and proto 15 as two
  half-B stores (scalar+sync) right off the vector engine. y is e-major
  [E, PL, B] fp16 so grouped stores have (g*512)-byte contiguous runs per
  partition row. No end-of-program store waits: the NEFF epilogue's
  per-engine DRAIN fences in-flight DMAs.
- Per-slot DMA-arrival semaphores are used because HWDGE completions of
  different DMAs can interleave (only per-slot counts are race-free).

The host reassembles cores' y into [B, E, P] f32.
"""

import os

import numpy as np

import concourse.bass as bass
from concourse import bacc, mybir
from concourse.bass_utils import run_bass_kernel_spmd

B, D, P, E = 256, 512, 128, 128
NCORES = 8
PL = P // NCORES  # prototypes per core
KC = D // 128  # contraction chunks of 128
CH = B + E  # 384, combined chunk width (x cols then w cols)
CW = KC * CH  # 1536 combined tile width

_nc_cache = None
LAST_RESULTS = None  # BassKernelResults of the most recent run (for test.py)

NB = 10  # combined xw sbuf ring depth (384 KiB fp16 each)
NPS = 8  # psum ring depth (8 banks)

# load plan: proto -> list of (ring, c_lo, c_hi) segments; rings: 0=sync 1=scalar
SEGS = {0: [(0, 0, 1), (1, 1, 4)], PL - 3: [(0, 0, 2), (0, 2, 4)],
        PL - 2: [(1, 0, 2), (1, 2, 4)],
        PL - 1: [(0, 0, 1), (0, 1, 2), (1, 2, 3), (1, 3, 4)]}
for _p in range(1, PL - 3):
    SEGS[_p] = [(_p % 2 == 0, 0, 4)]  # odd -> sync(0), even -> scalar(1)


def _build_nc() -> bass.Bass:
    nc = bacc.Bacc()
    mm_dt = mybir.dt.float16
    xw = nc.dram_tensor("xw", [PL, 128, CW], mm_dt, kind="ExternalInput")
    bT = nc.dram_tensor("bT", [E, PL], mybir.dt.float32, kind="ExternalInput")
    y = nc.dram_tensor("y", [E, PL, B], mm_dt, kind="ExternalOutput")

    buf = [
        nc.alloc_sbuf_tensor(f"buf{i}", [128, CW], mm_dt).ap() for i in range(NB)
    ]
    obuf = nc.alloc_sbuf_tensor("obuf", [E, PL * B], mm_dt).ap()
    pbuf = [
        nc.alloc_psum_tensor(f"pbuf{i}", [E, B], mybir.dt.float32).ap()
        for i in range(NPS)
    ]
    btile = nc.alloc_sbuf_tensor("btile", [E, PL], mybir.dt.float32).ap()
    # one arrival sem per load segment (slot reuse is serialized by the s_mm
    # guard, so per-segment counting is race-free even though HWDGE
    # completions interleave)
    seg_sem = {
        (p, si): nc.alloc_semaphore(f"s_p{p}s{si}")
        for p, segs in SEGS.items()
        for si in range(len(segs))
    }
    s_b = nc.alloc_semaphore("s_b")
    s_mm = nc.alloc_semaphore("s_mm")
    s_vec = nc.alloc_semaphore("s_vec")
    # walrus requires every DMA to carry a sync update (empty update list
    # SIGABRTs its codegen), so stores still bump a sem nobody waits on
    s_st = nc.alloc_semaphore("s_st")

    # PE wait for (p, c): the sem of the segment containing chunk c
    def chunk_sem(p, c):
        for si, (_, lo, hi) in enumerate(SEGS[p]):
            if lo <= c < hi:
                return seg_sem[(p, si)]
        raise AssertionError

    def emit_loads(eng, ring):
        for p in sorted(SEGS):
            for si, (r, lo, hi) in enumerate(SEGS[p]):
                if r != ring:
                    continue
                if p >= NB and si == 0:
                    eng.wait_ge(s_mm, p - NB + 1)
                eng.dma_start(
                    buf[p % NB][:, lo * CH : hi * CH], xw[p, :, lo * CH : hi * CH]
                ).then_inc(seg_sem[(p, si)], 16)

    def store(eng, p0, g, wait):
        eng.wait_ge(s_vec, wait)
        eng.dma_start(
            y[:, p0 : p0 + g, :], obuf[:, p0 * B : (p0 + g) * B]
        ).then_inc(s_st, 16)

    # "load rings drained" markers: the final segment sem on each ring
    last_seg = {0: (PL - 1, 1), 1: (PL - 1, 3)}

    with nc.Block() as block:

        @block.sync
        def _(sync: bass.BassEngine):
            emit_loads(sync, 0)
            # bulk store queues FIFO behind this ring's loads: it transfers
            # the moment the ring drains, with no idle gap and no extra gate
            store(sync, 0, 4, 4)  # protos 0-3
            store(sync, PL - 4, 2, PL - 2)  # protos 12-13
            # proto 15's second half-B as soon as vec wrote it
            sync.wait_ge(s_vec, PL + 1)
            sync.dma_start(
                y[:, PL - 1, B // 2 :],
                obuf[:, (PL - 1) * B + B // 2 : PL * B],
            ).then_inc(s_st, 16)

        @block.scalar
        def _(scalar: bass.BassEngine):
            emit_loads(scalar, 1)
            store(scalar, 4, 4, 8)  # protos 4-7
            store(scalar, PL - 2, 1, PL - 1)  # proto 14
            # proto 15's first half-B
            scalar.wait_ge(s_vec, PL)
            scalar.dma_start(
                y[:, PL - 1, : B // 2],
                obuf[:, (PL - 1) * B : (PL - 1) * B + B // 2],
            ).then_inc(s_st, 16)

        @block.tensor
        def _(tensor: bass.BassEngine):
            for p in range(PL):
                i = p % NB
                if p >= NPS:
                    tensor.wait_ge(s_vec, p - NPS + 1)
                seen = set()
                for c in range(KC):
                    sem = chunk_sem(p, c)
                    if sem.name not in seen:
                        seen.add(sem.name)
                        tensor.wait_ge(sem, 16)
                    mm = nc.tensor.matmul(
                        pbuf[p % NPS][:],
                        lhsT=buf[i][:, c * CH + B : (c + 1) * CH],
                        rhs=buf[i][:, c * CH : c * CH + B],
                        start=(c == 0),
                        stop=(c == KC - 1),
                    )
                mm.then_inc(s_mm, 1)

        @block.vector
        def _(vector: bass.BassEngine):
            vector.wait_ge(s_b, 16)
            for p in range(PL - 1):
                vector.wait_ge(s_mm, p + 1)
                nc.vector.tensor_scalar_add(
                    obuf[:, p * B : (p + 1) * B],
                    pbuf[p % NPS][:],
                    btile[:, p : p + 1],
                ).then_inc(s_vec, 1)
            # last proto in half-B pieces so each half-store launches early
            p = PL - 1
            vector.wait_ge(s_mm, p + 1)
            for h in range(2):
                nc.vector.tensor_scalar_add(
                    obuf[:, p * B + h * (B // 2) : p * B + (h + 1) * (B // 2)],
                    pbuf[p % NPS][:, h * (B // 2) : (h + 1) * (B // 2)],
                    btile[:, p : p + 1],
                ).then_inc(s_vec, 1)

        @block.gpsimd
        def _(gpsimd: bass.BassEngine):
            # bias rides the otherwise-idle SWDGE ring
            gpsimd.dma_start(btile[:], bT[:]).then_inc(s_b, 16)
            # hold this (separate SWDGE) queue's store until both load rings
            # have drained -- it would otherwise steal load bandwidth
            for ring in (0, 1):
                gpsimd.wait_ge(seg_sem[last_seg[ring]], 16)
            store(gpsimd, 8, 4, 12)  # protos 8-11

    nc.compile()
    return nc


def _shard_inputs(x: np.ndarray, W: np.ndarray, b: np.ndarray):
    x16 = x.astype(np.float16)
    w16 = W.astype(np.float16)
    # xk[p, k, c, b] = x[b, 128c + k, p]
    xk = x16.transpose(2, 1, 0).reshape(P, KC, 128, B).transpose(0, 2, 1, 3)
    # wk[p, k, c, e] = W[p, 128c + k, e]
    wk = w16.reshape(P, KC, 128, E).transpose(0, 2, 1, 3)
    # chunk-major pack: [P, 128, KC, B+E] -> [P, 128, CW]
    xwk = np.concatenate([xk, wk], axis=3).reshape(P, 128, CW)
    bT = b.T  # [E, P]
    in_maps = []
    for m in range(NCORES):
        sl = slice(m * PL, (m + 1) * PL)
        in_maps.append(
            {
                "xw": np.ascontiguousarray(xwk[sl]),
                "bT": np.ascontiguousarray(bT[:, sl]),
            }
        )
    return in_maps


def kernel(x: np.ndarray, W: np.ndarray, b: np.ndarray) -> np.ndarray:
    global _nc_cache, LAST_RESULTS
    x = np.asarray(x, dtype=np.float32)
    W = np.asarray(W, dtype=np.float32)
    b = np.ascontiguousarray(np.asarray(b, dtype=np.float32))
    if _nc_cache is None:
        _nc_cache = _build_nc()
    in_maps = _shard_inputs(x, W, b)
    # one retry: transient device wedges (NRT_EXEC_UNIT_UNRECOVERABLE) have
    # been observed on these shared cores and usually clear on re-execution
    try:
        res = run_bass_kernel_spmd(
            _nc_cache,
            in_maps,
            core_ids=list(range(NCORES)),
            trace=bool(os.environ.get("KERNEL_TRACE")),
        )
    except Exception:
        import time

        time.sleep(5)
        res = run_bass_kernel_spmd(
            _nc_cache,
            in_maps,
            core_ids=list(range(NCORES)),
            trace=False,
        )
    LAST_RESULTS = res
    # per-core y: [E, PL, B] fp16 -> full [E, P, B] -> out [B, E, P] f32
    yall = np.concatenate([r["y"] for r in res.results], axis=1)
    return np.ascontiguousarray(yall.transpose(2, 0, 1).astype(np.float32))


# revision 21
# speedup vs baseline: 1.0935x; 1.0935x over previous
"""Trainium2 Bass kernel for nn_Loop_Projection (batched per-prototype GEMM).

Computes out[b, e, p] = sum_d x[b, d, p] * W[p, d, e] + b[p, e] with
x: [256, 512, 128] f32, W: [128, 512, 128] f32, b: [128, 128] f32.

Sharding: prototype axis P=128 split across 8 NeuronCores (16 protos each).
Inputs are cast to fp16 on the host (well-scaled data: x ~ N(0,1),
W ~ U(+-0.107)), which halves HBM load traffic vs f32 and runs the PE at
full rate. Per proto, x and W slices are packed into ONE contiguous
[128, 1536] fp16 block, chunk-major so a DMA prefix is immediately usable:
  xw[p][k, 384c + b]       = x[b, 128c + k, p]     (b in [0,256))
  xw[p][k, 384c + 256 + e] = W[p, 128c + k, e]     (e in [0,128))

Schedule (per core, raw bacc, hand-placed semaphores):
- Loads ride the two HWDGE rings (SP=sync: odd protos, Act=scalar: even).
  DMA_DIRECT2D issue costs ~650 ns regardless of size, so mid-stream protos
  load as single 384 KiB DMAs. The first proto is split (96 KiB head on
  sync + 288 KiB on scalar) so the PE's first matmul starts ~2 us earlier,
  and the last three protos are split into halves/quarters so the PE tail
  tracks the final chunk arrivals instead of trailing whole-tile waits.
- Per proto the PE accumulates out.T = W_p.T @ x_p.T ([E, B] fp32 PSUM)
  over the 4 K-chunks; the vector engine adds bias during the PSUM->SBUF
  copy (casting to fp16) into one [E, PL*B] strip.
- Stores are DEFERRED until both load rings drain (loads and stores share
  the ~410 GB/s DMA-engine aggregate, so mid-stream stores push the
  critical last load later): protos 0-3/4-7/8-11 as three grouped DMAs on
  the gpsimd SWDGE ring, 12-13 on sync, 14 on scalar, and proto 15 as two
  half-B stores (scalar+sync) right off the vector engine. y is e-major
  [E, PL, B] fp16 so grouped stores have (g*512)-byte contiguous runs per
  partition row. No end-of-program store waits: the NEFF epilogue's
  per-engine DRAIN fences in-flight DMAs.
- Per-slot DMA-arrival semaphores are used because HWDGE completions of
  different DMAs can interleave (only per-slot counts are race-free).

The host reassembles cores' y into [B, E, P] f32.
"""

import os

import numpy as np

import concourse.bass as bass
from concourse import bacc, mybir
from concourse.bass_utils import run_bass_kernel_spmd

B, D, P, E = 256, 512, 128, 128
NCORES = 8
PL = P // NCORES  # prototypes per core
KC = D // 128  # contraction chunks of 128
CH = B + E  # 384, combined chunk width (x cols then w cols)
CW = KC * CH  # 1536 combined tile width

_nc_cache = None
LAST_RESULTS = None  # BassKernelResults of the most recent run (for test.py)

NB = 10  # combined xw sbuf ring depth (384 KiB fp16 each)
NPS = 8  # psum ring depth (8 banks)

# load plan: proto -> list of (ring, c_lo, c_hi) segments; rings: 0=sync 1=scalar
SEGS = {0: [(0, 0, 1), (1, 1, 4)], PL - 3: [(0, 0, 2), (0, 2, 4)],
        PL - 2: [(1, 0, 2), (1, 2, 4)],
        PL - 1: [(0, 0, 1), (0, 1, 2), (1, 2, 3), (1, 3, 4)]}
for _p in range(1, PL - 3):
    SEGS[_p] = [(_p % 2 == 0, 0, 4)]  # odd -> sync(0), even -> scalar(1)


def _build_nc() -> bass.Bass:
    nc = bacc.Bacc()
    mm_dt = mybir.dt.float16
    xw = nc.dram_tensor("xw", [PL, 128, CW], mm_dt, kind="ExternalInput")
    bT = nc.dram_tensor("bT", [E, PL], mybir.dt.float32, kind="ExternalInput")
    y = nc.dram_tensor("y", [E, PL, B], mm_dt, kind="ExternalOutput")

    buf = [
        nc.alloc_sbuf_tensor(f"buf{i}", [128, CW], mm_dt).ap() for i in range(NB)
    ]
    obuf = nc.alloc_sbuf_tensor("obuf", [E, PL * B], mm_dt).ap()
    pbuf = [
        nc.alloc_psum_tensor(f"pbuf{i}", [E, B], mybir.dt.float32).ap()
        for i in range(NPS)
    ]
    btile = nc.alloc_sbuf_tensor("btile", [E, PL], mybir.dt.float32).ap()
    # one arrival sem per load segment (slot reuse is serialized by the s_mm
    # guard, so per-segment counting is race-free even though HWDGE
    # completions interleave)
    seg_sem = {
        (p, si): nc.alloc_semaphore(f"s_p{p}s{si}")
        for p, segs in SEGS.items()
        for si in range(len(segs))
    }
    s_b = nc.alloc_semaphore("s_b")
    s_mm = nc.alloc_semaphore("s_mm")
    s_vec = nc.alloc_semaphore("s_vec")
    # walrus requires every DMA to carry a sync update (empty update list
    # SIGABRTs its codegen), so stores still bump a sem nobody waits on
    s_st = nc.alloc_semaphore("s_st")

    # PE wait for (p, c): the sem of the segment containing chunk c
    def chunk_sem(p, c):
        for si, (_, lo, hi) in enumerate(SEGS[p]):
            if lo <= c < hi:
                return seg_sem[(p, si)]
        raise AssertionError

    def emit_loads(eng, ring):
        for p in sorted(SEGS):
            for si, (r, lo, hi) in enumerate(SEGS[p]):
                if r != ring:
                    continue
                if p >= NB and si == 0:
                    eng.wait_ge(s_mm, p - NB + 1)
                eng.dma_start(
                    buf[p % NB][:, lo * CH : hi * CH], xw[p, :, lo * CH : hi * CH]
                ).then_inc(seg_sem[(p, si)], 16)

    def store(eng, p0, g, wait):
        eng.wait_ge(s_vec, wait)
        eng.dma_start(
            y[:, p0 : p0 + g, :], obuf[:, p0 * B : (p0 + g) * B]
        ).then_inc(s_st, 16)

    # "load rings drained" markers: the final segment sem on each ring
    last_seg = {0: (PL - 1, 1), 1: (PL - 1, 3)}

    with nc.Block() as block:

        @block.sync
        def _(sync: bass.BassEngine):
            emit_loads(sync, 0)
            # bulk store queues FIFO behind this ring's loads; gate on the
            # OTHER ring's drain so its transfer doesn't steal aggregate
            # bandwidth from that ring's final loads
            sync.wait_ge(seg_sem[last_seg[1]], 16)
            store(sync, 0, 4, 4)  # protos 0-3
            store(sync, PL - 4, 2, PL - 2)  # protos 12-13
            # proto 15's second half-B as soon as vec wrote it
            sync.wait_ge(s_vec, PL + 1)
            sync.dma_start(
                y[:, PL - 1, B // 2 :],
                obuf[:, (PL - 1) * B + B // 2 : PL * B],
            ).then_inc(s_st, 16)

        @block.scalar
        def _(scalar: bass.BassEngine):
            emit_loads(scalar, 1)
            scalar.wait_ge(seg_sem[last_seg[0]], 16)
            store(scalar, 4, 4, 8)  # protos 4-7
            store(scalar, PL - 2, 1, PL - 1)  # proto 14
            # proto 15's first half-B
            scalar.wait_ge(s_vec, PL)
            scalar.dma_start(
                y[:, PL - 1, : B // 2],
                obuf[:, (PL - 1) * B : (PL - 1) * B + B // 2],
            ).then_inc(s_st, 16)

        @block.tensor
        def _(tensor: bass.BassEngine):
            for p in range(PL):
                i = p % NB
                if p >= NPS:
                    tensor.wait_ge(s_vec, p - NPS + 1)
                seen = set()
                for c in range(KC):
                    sem = chunk_sem(p, c)
                    if sem.name not in seen:
                        seen.add(sem.name)
                        tensor.wait_ge(sem, 16)
                    mm = nc.tensor.matmul(
                        pbuf[p % NPS][:],
                        lhsT=buf[i][:, c * CH + B : (c + 1) * CH],
                        rhs=buf[i][:, c * CH : c * CH + B],
                        start=(c == 0),
                        stop=(c == KC - 1),
                    )
                mm.then_inc(s_mm, 1)

        @block.vector
        def _(vector: bass.BassEngine):
            vector.wait_ge(s_b, 16)
            for p in range(PL - 1):
                vector.wait_ge(s_mm, p + 1)
                nc.vector.tensor_scalar_add(
                    obuf[:, p * B : (p + 1) * B],
                    pbuf[p % NPS][:],
                    btile[:, p : p + 1],
                ).then_inc(s_vec, 1)
            # last proto in half-B pieces so each half-store launches early
            p = PL - 1
            vector.wait_ge(s_mm, p + 1)
            for h in range(2):
                nc.vector.tensor_scalar_add(
                    obuf[:, p * B + h * (B // 2) : p * B + (h + 1) * (B // 2)],
                    pbuf[p % NPS][:, h * (B // 2) : (h + 1) * (B // 2)],
                    btile[:, p : p + 1],
                ).then_inc(s_vec, 1)

        @block.gpsimd
        def _(gpsimd: bass.BassEngine):
            # bias rides the otherwise-idle SWDGE ring
            gpsimd.dma_start(btile[:], bT[:]).then_inc(s_b, 16)
            # hold this (separate SWDGE) queue's store until both load rings
            # have drained -- it would otherwise steal load bandwidth
            for ring in (0, 1):
                gpsimd.wait_ge(seg_sem[last_seg[ring]], 16)
            store(gpsimd, 8, 4, 12)  # protos 8-11

    nc.compile()
    return nc


def _shard_inputs(x: np.ndarray, W: np.ndarray, b: np.ndarray):
    x16 = x.astype(np.float16)
    w16 = W.astype(np.float16)
    # xk[p, k, c, b] = x[b, 128c + k, p]
    xk = x16.transpose(2, 1, 0).reshape(P, KC, 128, B).transpose(0, 2, 1, 3)
    # wk[p, k, c, e] = W[p, 128c + k, e]
    wk = w16.reshape(P, KC, 128, E).transpose(0, 2, 1, 3)
    # chunk-major pack: [P, 128, KC, B+E] -> [P, 128, CW]
    xwk = np.concatenate([xk, wk], axis=3).reshape(P, 128, CW)
    bT = b.T  # [E, P]
    in_maps = []
    for m in range(NCORES):
        sl = slice(m * PL, (m + 1) * PL)
        in_maps.append(
            {
                "xw": np.ascontiguousarray(xwk[sl]),
                "bT": np.ascontiguousarray(bT[:, sl]),
            }
        )
    return in_maps


def kernel(x: np.ndarray, W: np.ndarray, b: np.ndarray) -> np.ndarray:
    global _nc_cache, LAST_RESULTS
    x = np.asarray(x, dtype=np.float32)
    W = np.asarray(W, dtype=np.float32)
    b = np.ascontiguousarray(np.asarray(b, dtype=np.float32))
    if _nc_cache is None:
        _nc_cache = _build_nc()
    in_maps = _shard_inputs(x, W, b)
    # one retry: transient device wedges (NRT_EXEC_UNIT_UNRECOVERABLE) have
    # been observed on these shared cores and usually clear on re-execution
    try:
        res = run_bass_kernel_spmd(
            _nc_cache,
            in_maps,
            core_ids=list(range(NCORES)),
            trace=bool(os.environ.get("KERNEL_TRACE")),
        )
    except Exception:
        import time

        time.sleep(5)
        res = run_bass_kernel_spmd(
            _nc_cache,
            in_maps,
            core_ids=list(range(NCORES)),
            trace=False,
        )
    LAST_RESULTS = res
    # per-core y: [E, PL, B] fp16 -> full [E, P, B] -> out [B, E, P] f32
    yall = np.concatenate([r["y"] for r in res.results], axis=1)
    return np.ascontiguousarray(yall.transpose(2, 0, 1).astype(np.float32))
